# revision 29
# baseline (speedup 1.0000x reference)
"""Trainium2 Bass kernel for the MichaelsRNN forward pass.

Reference math (per time step t, per batch element b):
    recur = r @ J.T
    inp   = image.T @ I.T + hold.T * S.T
    pre   = 0.9*x + 0.1*(recur + inp + Bb.T)     # Euler step dt/tau = 1/10
    out   = retanh(pre) = tanh(max(pre, 0))
    y     = out[:, :100] @ fc_w.T + fc_b
    carry = (pre, out)

Sharding: data-parallel over the batch axis. B=1024 over 8 cores = 128
batch elements per core, further split into two phase-shifted
HALF-batches of 64: while PE runs half B's matmul group, ScalarE/VectorE
run half A's tanh/relu — the elementwise latency hides behind the other
half's PE block.

Per half-step, ONE PSUM accumulation group in one bank:
    9x J matmul      lhsT[122,100]=[0.1J[m,k].T ; k==0?[0.1I;0.1S;0.1Bb]_m:0]
                     rhs=rd_h[0:122, k]  (k0m0 opens the group)
    1x fc matmul     lhsT=[ysc*fc_w.T;0] [122,50] rhs=rd_h = y of t-1
                     (last, carries the stop)
The group MUST be opened by a matmul writing the region that later
accumulates — opening with the fc matmul (disjoint region) corrupts
the J accumulation.
The Euler step pre' = 0.9*pre + psum runs on DVE (tensor_scalar 0.9*pre
off the critical path, then tensor_tensor add from PSUM), which removes
the former identity matmuls AND the pre->PE dependency: PE's serial
input is only r (the DVE relu), cutting exec ~40%.
Elementwise: ACT tanh [100,192] (from SBUF); DVE relu via
tensor_tensor-max against a zero tile (2x mode), y bias add with int8
output (the int8 wire scale ysc=127/4 is folded into fc_w/fc_b).
Exec is PE-instruction-count bound (~20 matmuls + LDWs per step at
~70ns decode each); LDWEIGHTS itself overlaps matmuls (shadow buffer).

State per half (ping-pong on step parity to avoid WAR stalls):
    rd_{h,p} [122, 192]: rows 0:100 = r; rows 100:121 of module-slice 0 =
        the step's [image;hold] (DMA'd two steps ahead); row 121 slice 0 =
        ones (DMA'd once); rows 100:122 of slices 1,2 = zeros (memset
        once) — those rows only ever meet zero weights, so no host-side
        3x module broadcast of the data is needed.
y of step t-1 is computed inside step t's group (its input r_{t-1} is
still live then), so it costs no extra PSUM group.

Host I/O dominates wall-clock in this axon-tunneled setup (~25-45 MB/s
per direction, full duplex), so:
  - the runner keeps persistent jitted shard_map callables (re-tracing
    per call costs seconds);
  - wire formats are dieted: data H2D bf16 [21, T*128] per core with no
    module broadcast, weights packed into two replicated arrays, y D2H
    int8 (scale folded into fc);
  - the sequence is split into phases with the carried state (pre, r) as
    device-resident tensors between the per-phase NEFFs, so phase k+1's
    data upload and execution overlap phase k's y download;
  - y shards convert (int8 -> f32 transpose) on worker threads while the
    next shard downloads.
"""

import numpy as np
import ml_dtypes

import concourse.bass as bass  # noqa: F401
import concourse.tile as tile
from concourse import bacc, mybir

NPM = 100
NMOD = 3
NN = 300
NF = 20
OUT = 50
T = 500
B = 1024
N_CORES = 8
BS = B // N_CORES      # 128 batch per core
NH = 2                 # phase-shifted half-batches
HB = BS // NH          # 64
HFREE = NMOD * HB      # 192
KDATA = NF + 1         # 21 data rows on the wire (image, hold)
KD = KDATA + 1         # 22 data rows in SBUF (plus ones)
KJ = NPM + KD          # 122
CH = 20                # steps per y-out chunk
SFREE = NH * HFREE     # 384: state tensors hold both halves

W_DT = "bf16"
D_DT = "int8"          # data wire format: "int8" (dynamic scale folded into
                       # the I/S weight rows; int8->bf16 convert on ACT via a
                       # staging tile since DMA can't convert and compute
                       # engines can't write at partition 100) or "bf16"
Y_DT = "int8"          # y wire format: "int8" (scale folded into fc) or "bf16"
Y_SCALE = 127.0 / 4.0  # int8 quantization: q = clip(round(y*Y_SCALE)); |y| < 4
PHASES = 1             # >1 pipelines H2D under D2H; dispatch overhead (~70ms
                       # per phase) cancels the gain on this tunnel, so keep 1
MIN_PHASE_STEPS = 50

W16_JT = 9 * NPM                    # col offsets inside the w16 pack
W16_FCT = W16_JT
W16_ONES = W16_FCT + OUT
W16_COLS = W16_ONES + HB            # jt | fct | ones row

_BUILD_CACHE: dict = {}
_RUNNER_CACHE: dict = {}


def _w_np():
    return ml_dtypes.bfloat16 if W_DT == "bf16" else np.float32


def _w_mybir():
    return mybir.dt.bfloat16 if W_DT == "bf16" else mybir.dt.float32


def _build_program(n_steps: int, n_repeat: int = 1, variant: str = "full"):
    """Build + compile the Bass program (value-independent).

    The program runs n_steps of the recurrence from state (sin_pre,
    sin_r) and emits y plus the final state (sout_pre, sout_r), so a
    full sequence can be pipelined as several shorter phases.

    n_repeat re-runs the phase on-device via tc.For_i (state
    re-initialized from DRAM each iteration, y overwritten identically)
    — used for timing via wall-clock deltas.
    """
    wdt = _w_mybir()
    f32 = mybir.dt.float32
    import contextlib

    nc = bacc.Bacc(
        "TRN2", target_bir_lowering=False, debug=False, num_devices=N_CORES
    )

    ddt = mybir.dt.int8 if D_DT == "int8" else wdt
    # din: [21, (t, b128)] — per (t,h) slab is cols t*BS+h*HB, width HB
    din_ap = nc.dram_tensor(
        "din", [KDATA, n_steps * BS], ddt, kind="ExternalInput"
    ).ap()
    w16_ap = nc.dram_tensor(
        "w16", [KJ, W16_COLS], wdt, kind="ExternalInput"
    ).ap()
    w32_ap = nc.dram_tensor(
        "w32", [NPM, 1], f32, kind="ExternalInput"
    ).ap()
    sin_pre_ap = nc.dram_tensor(
        "sin_pre", [NPM, SFREE], f32, kind="ExternalInput"
    ).ap()
    ydt = mybir.dt.int8 if Y_DT == "int8" else wdt
    y_ap = nc.dram_tensor(
        "y", [OUT, n_steps * BS], ydt, kind="ExternalOutput"
    ).ap()
    sout_pre_ap = nc.dram_tensor(
        "sout_pre", [NPM, SFREE], f32, kind="ExternalOutput"
    ).ap()

    ch = min(CH, n_steps)

    def dslice(t, h):
        off = t * BS + h * HB
        return din_ap[:, off : off + HB]

    def hs(ap, h):
        return ap[:, h * HFREE : (h + 1) * HFREE]

    with tile.TileContext(nc) as tc:
        with contextlib.ExitStack() as ctx:
            const_pool = ctx.enter_context(tc.tile_pool(name="const", bufs=1))
            yout_pool = ctx.enter_context(tc.tile_pool(name="yout", bufs=2))
            tmp_pool = ctx.enter_context(tc.tile_pool(name="tmp", bufs=2))
            ps_pool = ctx.enter_context(
                tc.tile_pool(name="ps", bufs=2, space="PSUM")
            )

            def stage_data(t, h, rd_tile):
                # d(t,h) -> rd rows 100:121 of module-slice 0. With int8
                # wire data: DMA to an int8 staging tile, ACT-convert to
                # bf16, then SBUF->SBUF DMA into rd (compute engines can't
                # write at partition 100; DMA can't convert).
                if D_DT == "int8":
                    s8 = tmp_pool.tile([KDATA, HB], ddt, tag=f"s8{h}")
                    s16 = tmp_pool.tile([KDATA, HB], wdt, tag=f"s16{h}")
                    nc.sync.dma_start(s8[:], dslice(t, h))
                    nc.scalar.copy(s16[:], s8[:])
                    nc.sync.dma_start(
                        rd_tile[NPM : NPM + KDATA, 0:HB], s16[:]
                    )
                else:
                    nc.sync.dma_start(
                        rd_tile[NPM : NPM + KDATA, 0:HB], dslice(t, h)
                    )

            jt = const_pool.tile([KJ, 9 * NPM], wdt)
            nc.sync.dma_start(jt[:], w16_ap[:, 0 : 9 * NPM])
            fct = const_pool.tile([KJ, OUT], wdt)
            nc.sync.dma_start(fct[:], w16_ap[:, W16_FCT : W16_FCT + OUT])
            fcb = const_pool.tile([OUT, 1], f32)
            nc.sync.dma_start(fcb[:], w32_ap[0:OUT, 0:1])
            zeros = const_pool.tile([NPM, HFREE], wdt)
            nc.vector.memset(zeros[:], 0.0)

            pre_a0 = const_pool.tile([NPM, HFREE], f32)
            pre_a1 = const_pool.tile([NPM, HFREE], f32)
            pre_b0 = const_pool.tile([NPM, HFREE], f32)
            pre_b1 = const_pool.tile([NPM, HFREE], f32)
            pres = [[pre_a0, pre_a1], [pre_b0, pre_b1]]
            rd_a0 = const_pool.tile([KJ, HFREE], wdt)
            rd_a1 = const_pool.tile([KJ, HFREE], wdt)
            rd_b0 = const_pool.tile([KJ, HFREE], wdt)
            rd_b1 = const_pool.tile([KJ, HFREE], wdt)
            rds = [[rd_a0, rd_a1], [rd_b0, rd_b1]]
            # data rows that only ever meet zero weights: zero the whole
            # tile once (memset must start at partition 0); the ones row
            # (drives Bb) in module-slice 0 arrives by DMA (no partition-
            # start restriction).
            for h in range(NH):
                for p in range(2):
                    nc.vector.memset(rds[h][p][:], 0.0)
                    nc.sync.dma_start(
                        rds[h][p][KJ - 1 : KJ, 0:HB],
                        w16_ap[0:1, W16_ONES : W16_ONES + HB],
                    )
            if variant in ("no_chain", "ew_only"):
                dump_r = const_pool.tile([NPM, HFREE], wdt)
                dump_p = const_pool.tile([NPM, HFREE], f32)
            if variant == "ew_only":
                psc_pool = ctx.enter_context(
                    tc.tile_pool(name="psc", bufs=1, space="PSUM")
                )
                ew_ps0 = psc_pool.tile([128, 512], f32)
                ew_ps1 = psc_pool.tile([128, 512], f32)
                nc.vector.memset(ew_ps0[:], 0.25)
                nc.vector.memset(ew_ps1[:], 0.25)
                ew_pss = [ew_ps0, ew_ps1]

            rep_ctx = (
                tc.For_i(0, n_repeat, 1)
                if n_repeat > 1
                else contextlib.nullcontext()
            )
            with rep_ctx:
                for h in range(NH):
                    nc.sync.dma_start(pres[h][0][:], hs(sin_pre_ap, h))
                    # r = retanh(pre) identically, so only pre is carried
                    # state; recompute r here
                    th0 = tmp_pool.tile([NPM, HFREE], wdt, tag=f"init{h}")
                    nc.scalar.activation(
                        th0[:], pres[h][0][:],
                        mybir.ActivationFunctionType.Tanh,
                    )
                    nc.vector.tensor_tensor(
                        rds[h][0][0:NPM, :], th0[:], zeros[:],
                        op=mybir.AluOpType.max,
                    )
                    stage_data(0, h, rds[h][0])
                    if n_steps > 1:
                        stage_data(1, h, rds[h][1])
                    if variant in ("no_chain", "pe_only"):
                        nc.vector.tensor_tensor(
                            rds[h][1][0:NPM, :], th0[:], zeros[:],
                            op=mybir.AluOpType.max,
                        )

                ybuf = None
                for t in range(n_steps):
                    s = t - 1          # step whose y this group computes
                    if s % ch == 0:
                        ybuf = yout_pool.tile([OUT, ch * BS], ydt, tag="ybuf")
                    for h in range(NH):
                        pre_cur = pres[h][t % 2]
                        pre_nxt = pres[h][(t + 1) % 2]
                        rd = rds[h][t % 2]
                        rd_nxt = rds[h][(t + 1) % 2]

                        if variant == "ew_only":
                            ps = ew_pss[h]
                        else:
                            ps = ps_pool.tile([128, 512], f32, tag=f"ps{h}")
                        for k in range(NMOD):
                            if variant == "ew_only":
                                break
                            rk = rd[0:KJ, k * HB : (k + 1) * HB]
                            for m in range(NMOD):
                                nc.tensor.matmul(
                                    ps[0:NPM, m * HB : (m + 1) * HB],
                                    jt[:, (k * NMOD + m) * NPM : (k * NMOD + m) * NPM + NPM],
                                    rk,
                                    start=(k == 0 and m == 0),
                                    stop=False,
                                )
                        # y_{t-1}: r_{t-1} is rd's r rows (relu_t writes
                        # rd_nxt, not rd). Last in the group, carries stop.
                        if variant != "ew_only":
                            nc.tensor.matmul(
                                ps[0:OUT, HFREE : HFREE + HB],
                                fct[:],
                                rd[0:KJ, 0:HB],
                                start=False,
                                stop=True,
                            )
                        # --- elementwise (overlaps the other half's PE) ---
                        if variant == "pe_only":
                            if t + 2 < n_steps:
                                stage_data(t + 2, h, rd)
                            continue
                        # Euler step on DVE: pre' = 0.9*pre + 0.1*(...)
                        # (the 0.1 is folded into jt; saves PE the ident
                        # matmuls + their LDWEIGHTS). The 0.9*pre scaling
                        # runs off the critical path (no PSUM dependency).
                        th = tmp_pool.tile([NPM, HFREE], wdt, tag=f"th{h}")
                        sc = tmp_pool.tile([NPM, HFREE], f32, tag=f"sc{h}")
                        if variant in ("no_chain", "ew_only"):
                            nc.vector.tensor_scalar_mul(
                                sc[:], pres[h][0][:], 0.9
                            )
                            nc.vector.tensor_tensor(
                                dump_p[:], sc[:], ps[0:NPM, 0:HFREE],
                                op=mybir.AluOpType.add,
                            )
                            nc.scalar.activation(
                                th[:], dump_p[:],
                                mybir.ActivationFunctionType.Tanh,
                            )
                            nc.vector.tensor_tensor(
                                dump_r[:], th[:], zeros[:],
                                op=mybir.AluOpType.max,
                            )
                        else:
                            nc.vector.tensor_scalar_mul(
                                sc[:], pre_cur[:], 0.9
                            )
                            nc.vector.tensor_tensor(
                                pre_nxt[:], sc[:], ps[0:NPM, 0:HFREE],
                                op=mybir.AluOpType.add,
                            )
                            nc.scalar.activation(
                                th[:], pre_nxt[:],
                                mybir.ActivationFunctionType.Tanh,
                            )
                            # r <- relu(tanh) via TT-max (2x DVE mode)
                            nc.vector.tensor_tensor(
                                rd_nxt[0:NPM, :], th[:], zeros[:],
                                op=mybir.AluOpType.max,
                            )
                        if t > 0:
                            nc.vector.tensor_scalar_add(
                                ybuf[:, (s % ch) * BS + h * HB : (s % ch) * BS + (h + 1) * HB],
                                ps[0:OUT, HFREE : HFREE + HB],
                                fcb[:],
                            )
                        # stage d_{t+2} for this parity tile (WAR: this
                        # group's J matmuls; ~2 steps of slack).
                        if t + 2 < n_steps:
                            stage_data(t + 2, h, rd)
                    if variant != "pe_only" and t > 0 and s % ch == ch - 1:
                        nc.sync.dma_start(
                            y_ap[:, (s - ch + 1) * BS : (s + 1) * BS], ybuf[:]
                        )

                # trailing: y of the last step, per half
                s = n_steps - 1
                if s % ch == 0:
                    ybuf = yout_pool.tile([OUT, ch * BS], ydt, tag="ybuf")
                for h in range(NH):
                    ps = ps_pool.tile([128, 512], f32, tag=f"ps{h}")
                    nc.tensor.matmul(
                        ps[0:OUT, HFREE : HFREE + HB],
                        fct[:],
                        rds[h][n_steps % 2][0:KJ, 0:HB],
                        start=True,
                        stop=True,
                    )
                    nc.vector.tensor_scalar_add(
                        ybuf[:, (s % ch) * BS + h * HB : (s % ch) * BS + (h + 1) * HB],
                        ps[0:OUT, HFREE : HFREE + HB],
                        fcb[:],
                    )
                nc.sync.dma_start(
                    y_ap[:, (s - s % ch) * BS : (s + 1) * BS],
                    ybuf[:, : (s % ch + 1) * BS],
                )

                # final state out (for phase pipelining)
                for h in range(NH):
                    nc.sync.dma_start(hs(sout_pre_ap, h), pres[h][n_steps % 2][:])

    nc.compile()
    return nc


def _get_program(n_steps: int, n_repeat: int = 1, variant: str = "full"):
    key = (n_steps, W_DT, Y_DT, n_repeat, NH, variant)
    if key not in _BUILD_CACHE:
        _BUILD_CACHE[key] = _build_program(n_steps, n_repeat, variant)
    return _BUILD_CACHE[key]


def _plan_chunks(n_steps: int):
    phases = PHASES
    while phases > 1 and n_steps < phases * MIN_PHASE_STEPS:
        phases -= 1
    base = n_steps // phases
    rem = n_steps - base * phases
    # equal chunks when divisible (one compiled program serves all phases)
    return [base + (1 if i < rem else 0) for i in range(phases)]


def _rep8(a):
    return np.ascontiguousarray(
        np.broadcast_to(a, (N_CORES, *a.shape))
    ).reshape(N_CORES * a.shape[0], a.shape[1])


def _prep_arrays(data, J, I, S, Bb, x0, fc_w, fc_b, chunks):
    """Build the global (axis-0 concatenated) input arrays for shard_map."""
    wnp = _w_np()
    f32 = np.float32

    n_steps = sum(chunks)
    dat_f = np.asarray(data, f32)[:n_steps]
    if D_DT == "int8":
        # amax via two reductions (no 43MB abs temporary)
        amax = max(float(dat_f.max()), -float(dat_f.min()))
        dsc = np.float32(max(amax / 127.0, 1e-30))
    else:
        dsc = np.float32(1.0)

    Jp = 0.1 * np.asarray(J, f32)
    Ip = 0.1 * dsc * np.asarray(I, f32)   # data wire scale rides the weights
    Sp = 0.1 * dsc * np.asarray(S, f32)
    Bbp = 0.1 * np.asarray(Bb, f32)       # Bb enters via the ones row: unscaled

    # jt: rows 0:100 = J'[m,k].T ; rows 100:122 = input weights on k==0
    jt = np.zeros((KJ, 9, NPM), f32)
    for k in range(NMOD):
        for m in range(NMOD):
            blk = Jp[m * NPM : (m + 1) * NPM, k * NPM : (k + 1) * NPM]
            jt[:NPM, k * NMOD + m, :NPM] = blk.T
            if k == 0:
                jt[NPM : NPM + NF, k * NMOD + m, :NPM] = (
                    Ip[m * NPM : (m + 1) * NPM, :].T
                )
                jt[NPM + NF, k * NMOD + m, :NPM] = Sp[m * NPM : (m + 1) * NPM, 0]
                jt[NPM + NF + 1, k * NMOD + m, :NPM] = (
                    Bbp[m * NPM : (m + 1) * NPM, 0]
                )

    ysc = Y_SCALE if Y_DT == "int8" else 1.0  # y wire scale folds into fc
    w16 = np.zeros((KJ, W16_COLS), f32)
    w16[:, : 9 * NPM] = jt.reshape(KJ, 9 * NPM)
    w16[:NPM, W16_FCT : W16_FCT + OUT] = ysc * np.asarray(fc_w, f32).T
    w16[0, W16_ONES : W16_ONES + HB] = 1.0
    w16 = w16.astype(wnp)

    w32 = np.zeros((NPM, 1), f32)
    w32[:OUT, 0] = ysc * np.asarray(fc_b, f32)

    x0 = np.asarray(x0, f32)
    pre0 = np.repeat(
        x0.reshape(NMOD, NPM).T[:, :, None], HB, axis=2
    ).reshape(NPM, HFREE)
    sin_pre = np.tile(pre0, (1, NH))                # same state in both halves

    # din: per phase [8*21, steps*128] — core-major, then t-major
    if D_DT == "int8":
        # |x|/dsc <= 127 by construction of dsc, so no clip needed;
        # rint == round (both half-even), minus round()'s overhead
        tmp = dat_f * np.float32(1.0 / dsc)
        np.rint(tmp, out=tmp)
        dat = tmp.astype(np.int8)
    else:
        dat = dat_f.astype(wnp)           # [T, 21, B]
    dins, t0 = [], 0
    for c in chunks:
        dins.append(
            np.ascontiguousarray(
                np.transpose(
                    dat[t0 : t0 + c].reshape(c, KDATA, N_CORES, BS),
                    (2, 1, 0, 3),
                )
            ).reshape(N_CORES * KDATA, c * BS)
        )
        t0 += c

    return {
        "din": dins,
        "w16": _rep8(w16),
        "w32": _rep8(w32),
        "sin_pre": _rep8(sin_pre),
    }


class _Runner:
    """Persistent jitted shard_map callable for one compiled program."""

    IN_ORDER = ("din", "w16", "w32", "sin_pre")

    def __init__(self, nc):
        import jax
        import jax.numpy as jnp
        from jax.sharding import Mesh, PartitionSpec
        from jax.experimental.shard_map import shard_map
        from concourse.bass2jax import (
            _bass_exec_p,
            install_neuronx_cc_hook,
            partition_id_tensor,
        )

        install_neuronx_cc_hook()
        self.nc = nc
        partition_name = (
            nc.partition_id_tensor.name if nc.partition_id_tensor else None
        )

        in_names, out_names, out_avals, zero_shapes = [], [], [], []
        for alloc in nc.m.functions[0].allocations:
            if not isinstance(alloc, mybir.MemoryLocationSet):
                continue
            name = alloc.memorylocations[0].name
            if alloc.kind == "ExternalInput":
                if name != partition_name:
                    in_names.append(name)
            elif alloc.kind == "ExternalOutput":
                np_dt = mybir.dt.np(alloc.dtype)
                out_avals.append(
                    jax.core.ShapedArray(tuple(alloc.tensor_shape), np_dt)
                )
                out_names.append(name)
                zero_shapes.append((tuple(alloc.tensor_shape), np_dt))
        assert tuple(in_names) == self.IN_ORDER, in_names
        assert out_names[0] == "y", out_names
        self.in_names = in_names
        self.out_names = out_names

        n_params = len(in_names)
        n_outs = len(out_names)
        all_in_names = list(in_names) + list(out_names)
        if partition_name is not None:
            all_in_names.append(partition_name)

        def _body(*args):
            operands = list(args)
            if partition_name is not None:
                operands.append(partition_id_tensor())
            outs = _bass_exec_p.bind(
                *operands,
                out_avals=tuple(out_avals),
                in_names=tuple(all_in_names),
                out_names=tuple(out_names),
                lowering_input_output_aliases=(),
                sim_require_finite=True,
                sim_require_nnan=True,
                nc=nc,
            )
            return tuple(outs)

        devices = jax.devices()[:N_CORES]
        mesh = Mesh(np.asarray(devices), ("core",))
        in_specs = (PartitionSpec("core"),) * (n_params + n_outs)
        out_specs = (PartitionSpec("core"),) * n_outs
        self.mesh = mesh
        self.sharded = jax.jit(
            shard_map(
                _body, mesh=mesh, in_specs=in_specs, out_specs=out_specs,
                check_rep=False,
            ),
            keep_unused=True,
        )
        # device-resident zero output buffers, reused every call
        self.zeros = [
            jnp.zeros((N_CORES * shp[0], *shp[1:]), dt)
            for shp, dt in zero_shapes
        ]

    def __call__(self, din, w16, w32, sin_pre):
        return self.sharded(din, w16, w32, sin_pre, *self.zeros)


def _get_runner(n_steps: int, n_repeat: int = 1, variant: str = "full"):
    key = (n_steps, W_DT, Y_DT, n_repeat, NH, variant)
    if key not in _RUNNER_CACHE:
        _RUNNER_CACHE[key] = _Runner(_get_program(n_steps, n_repeat, variant))
    return _RUNNER_CACHE[key]


def _convert_shard(dst_f32, qa, c, n_steps, t_off):
    dst = dst_f32[t_off : t_off + n_steps, c * BS : (c + 1) * BS, :]
    if Y_DT == "int8":
        v = qa.view(np.int8).reshape(OUT, n_steps, BS).transpose(1, 2, 0)
        np.multiply(
            v, np.float32(1.0 / Y_SCALE), out=dst, casting="unsafe"
        )
    else:
        v = qa.view(ml_dtypes.bfloat16).reshape(OUT, n_steps, BS)
        dst[...] = v.transpose(1, 2, 0)


def _gather_y(y_global: np.ndarray, n_steps: int) -> np.ndarray:
    """[8*OUT, n_steps*BS] wire format -> [n_steps, B, OUT] f32."""
    final = np.empty((n_steps, B, OUT), np.float32)
    per = np.asarray(y_global).reshape(N_CORES, OUT, n_steps * BS)
    for c in range(N_CORES):
        _convert_shard(final, per[c], c, n_steps, 0)
    return final


def run_sharded(inputs: dict, n_steps: int = T):
    """Compile (cached), run on 8 cores phase-pipelined, return [T, B, OUT]."""
    from concurrent.futures import ThreadPoolExecutor
    import jax
    from jax.sharding import NamedSharding, PartitionSpec

    chunks = _plan_chunks(n_steps)
    runners = [_get_runner(c) for c in chunks]
    arrays = _prep_arrays(chunks=chunks, **inputs)

    # multi-phase: weights ride the tunnel once, then stay device-resident
    if len(chunks) > 1:
        spec = NamedSharding(runners[0].mesh, PartitionSpec("core"))
        w16 = jax.device_put(arrays["w16"], spec)
        w32 = jax.device_put(arrays["w32"], spec)
    else:
        w16, w32 = arrays["w16"], arrays["w32"]

    ys = []
    state = arrays["sin_pre"]
    for i, c in enumerate(chunks):
        outs = runners[i](arrays["din"][i], w16, w32, state)
        ys.append(outs[0])
        state = outs[1]

    # all phases dispatched async; the tunnel serializes shard fetches,
    # so convert each shard on a worker thread while the next downloads
    final = np.empty((n_steps, B, OUT), np.float32)
    with ThreadPoolExecutor(2) as ex:
        futs, t_off = [], 0
        for i, c in enumerate(chunks):
            shards = sorted(
                ys[i].addressable_shards, key=lambda s: s.index[0].start
            )
            for sh in shards:
                sh.data.copy_to_host_async()
            for cc, sh in enumerate(shards):
                futs.append(
                    ex.submit(
                        _convert_shard, final, np.asarray(sh.data), cc, c, t_off
                    )
                )
            t_off += c
        for f in futs:
            f.result()
    return final


def kernel(data, J, I, S, Bb, x0, fc_w, fc_b):
    return run_sharded(
        dict(data=data, J=J, I=I, S=S, Bb=Bb, x0=x0, fc_w=fc_w, fc_b=fc_b)
    )


# revision 30
# speedup vs baseline: 1.1087x; 1.1087x over previous
"""Trainium2 Bass kernel for the MichaelsRNN forward pass.

Reference math (per time step t, per batch element b):
    recur = r @ J.T
    inp   = image.T @ I.T + hold.T * S.T
    pre   = 0.9*x + 0.1*(recur + inp + Bb.T)     # Euler step dt/tau = 1/10
    out   = retanh(pre) = tanh(max(pre, 0))
    y     = out[:, :100] @ fc_w.T + fc_b
    carry = (pre, out)

Sharding: data-parallel over the batch axis. B=1024 over 8 cores = 128
batch elements per core, further split into two phase-shifted
HALF-batches of 64: while PE runs half B's matmul group, ScalarE/VectorE
run half A's tanh/relu — the elementwise latency hides behind the other
half's PE block.

Per half-step, ONE PSUM accumulation group in one bank:
    9x J matmul      lhsT[122,100]=[0.1J[m,k].T ; k==0?[0.1I;0.1S;0.1Bb]_m:0]
                     rhs=rd_h[0:122, k]  (k0m0 opens the group)
    1x fc matmul     lhsT=[ysc*fc_w.T;0] [122,50] rhs=rd_h = y of t-1
                     (last, carries the stop)
The group MUST be opened by a matmul writing the region that later
accumulates — opening with the fc matmul (disjoint region) corrupts
the J accumulation.
The Euler step pre' = 0.9*pre + psum runs on DVE (tensor_scalar 0.9*pre
off the critical path, then tensor_tensor add from PSUM), which removes
the former identity matmuls AND the pre->PE dependency: PE's serial
input is only r (the DVE relu), cutting exec ~40%.
Elementwise: ACT tanh [100,192] (from SBUF); DVE relu via
tensor_tensor-max against a zero tile (2x mode), y bias add with int8
output (the int8 wire scale ysc=127/4 is folded into fc_w/fc_b).
Exec is PE-instruction-count bound (~20 matmuls + LDWs per step at
~70ns decode each); LDWEIGHTS itself overlaps matmuls (shadow buffer).

State per half (ping-pong on step parity to avoid WAR stalls):
    rd_{h,p} [122, 192]: rows 0:100 = r; rows 100:121 of module-slice 0 =
        the step's [image;hold] (DMA'd two steps ahead); row 121 slice 0 =
        ones (DMA'd once); rows 100:122 of slices 1,2 = zeros (memset
        once) — those rows only ever meet zero weights, so no host-side
        3x module broadcast of the data is needed.
y of step t-1 is computed inside step t's group (its input r_{t-1} is
still live then), so it costs no extra PSUM group.

Host I/O dominates wall-clock in this axon-tunneled setup (~25-45 MB/s
per direction, full duplex), so:
  - the runner keeps persistent jitted shard_map callables (re-tracing
    per call costs seconds);
  - wire formats are dieted: data H2D bf16 [21, T*128] per core with no
    module broadcast, weights packed into two replicated arrays, y D2H
    int8 (scale folded into fc);
  - the sequence is split into phases with the carried state (pre, r) as
    device-resident tensors between the per-phase NEFFs, so phase k+1's
    data upload and execution overlap phase k's y download;
  - y shards convert (int8 -> f32 transpose) on worker threads while the
    next shard downloads.
"""

import numpy as np
import ml_dtypes

import concourse.bass as bass  # noqa: F401
import concourse.tile as tile
from concourse import bacc, mybir

NPM = 100
NMOD = 3
NN = 300
NF = 20
OUT = 50
T = 500
B = 1024
N_CORES = 8
BS = B // N_CORES      # 128 batch per core
NH = 2                 # phase-shifted half-batches
HB = BS // NH          # 64
HFREE = NMOD * HB      # 192
KDATA = NF + 1         # 21 data rows on the wire (image, hold)
KD = KDATA + 1         # 22 data rows in SBUF (plus ones)
KJ = NPM + KD          # 122
CH = 20                # steps per y-out chunk
SFREE = NH * HFREE     # 384: state tensors hold both halves

W_DT = "bf16"
D_DT = "int8"          # data wire format: "int8" (dynamic scale folded into
                       # the I/S weight rows; int8->bf16 convert on ACT via a
                       # staging tile since DMA can't convert and compute
                       # engines can't write at partition 100) or "bf16"
Y_DT = "int8"          # y wire format: "int8" (scale folded into fc) or "bf16"
Y_SCALE = 127.0 / 4.0  # int8 quantization: q = clip(round(y*Y_SCALE)); |y| < 4
PHASES = 1             # >1 pipelines H2D under D2H; dispatch overhead (~70ms
                       # per phase) cancels the gain on this tunnel, so keep 1
MIN_PHASE_STEPS = 50

W16_JT = 9 * NPM                    # col offsets inside the w16 pack
W16_FCT = W16_JT
W16_ONES = W16_FCT + OUT
W16_COLS = W16_ONES + HB            # jt | fct | ones row

_BUILD_CACHE: dict = {}
_RUNNER_CACHE: dict = {}


def _w_np():
    return ml_dtypes.bfloat16 if W_DT == "bf16" else np.float32


def _w_mybir():
    return mybir.dt.bfloat16 if W_DT == "bf16" else mybir.dt.float32


def _build_program(n_steps: int, n_repeat: int = 1, variant: str = "full"):
    """Build + compile the Bass program (value-independent).

    The program runs n_steps of the recurrence from state (sin_pre,
    sin_r) and emits y plus the final state (sout_pre, sout_r), so a
    full sequence can be pipelined as several shorter phases.

    n_repeat re-runs the phase on-device via tc.For_i (state
    re-initialized from DRAM each iteration, y overwritten identically)
    — used for timing via wall-clock deltas.
    """
    wdt = _w_mybir()
    f32 = mybir.dt.float32
    import contextlib

    nc = bacc.Bacc(
        "TRN2", target_bir_lowering=False, debug=False, num_devices=N_CORES
    )

    ddt = mybir.dt.int8 if D_DT == "int8" else wdt
    # din: [21, (t, b128)] — per (t,h) slab is cols t*BS+h*HB, width HB
    din_ap = nc.dram_tensor(
        "din", [KDATA, n_steps * BS], ddt, kind="ExternalInput"
    ).ap()
    w16_ap = nc.dram_tensor(
        "w16", [KJ, W16_COLS], wdt, kind="ExternalInput"
    ).ap()
    w32_ap = nc.dram_tensor(
        "w32", [NPM, 1 + NMOD], f32, kind="ExternalInput"
    ).ap()
    ydt = mybir.dt.int8 if Y_DT == "int8" else wdt
    y_ap = nc.dram_tensor(
        "y", [OUT, n_steps * BS], ydt, kind="ExternalOutput"
    ).ap()
    sout_pre_ap = nc.dram_tensor(
        "sout_pre", [NPM, SFREE], f32, kind="ExternalOutput"
    ).ap()

    ch = min(CH, n_steps)

    def dslice(t, h):
        off = t * BS + h * HB
        return din_ap[:, off : off + HB]

    def hs(ap, h):
        return ap[:, h * HFREE : (h + 1) * HFREE]

    with tile.TileContext(nc) as tc:
        with contextlib.ExitStack() as ctx:
            const_pool = ctx.enter_context(tc.tile_pool(name="const", bufs=1))
            yout_pool = ctx.enter_context(tc.tile_pool(name="yout", bufs=2))
            tmp_pool = ctx.enter_context(tc.tile_pool(name="tmp", bufs=2))
            ps_pool = ctx.enter_context(
                tc.tile_pool(name="ps", bufs=2, space="PSUM")
            )

            def stage_data(t, h, rd_tile):
                # d(t,h) -> rd rows 100:121 of module-slice 0. With int8
                # wire data: DMA to an int8 staging tile, ACT-convert to
                # bf16, then SBUF->SBUF DMA into rd (compute engines can't
                # write at partition 100; DMA can't convert).
                if D_DT == "int8":
                    s8 = tmp_pool.tile([KDATA, HB], ddt, tag=f"s8{h}")
                    s16 = tmp_pool.tile([KDATA, HB], wdt, tag=f"s16{h}")
                    nc.sync.dma_start(s8[:], dslice(t, h))
                    nc.scalar.copy(s16[:], s8[:])
                    nc.sync.dma_start(
                        rd_tile[NPM : NPM + KDATA, 0:HB], s16[:]
                    )
                else:
                    nc.sync.dma_start(
                        rd_tile[NPM : NPM + KDATA, 0:HB], dslice(t, h)
                    )

            jt = const_pool.tile([KJ, 9 * NPM], wdt)
            nc.sync.dma_start(jt[:], w16_ap[:, 0 : 9 * NPM])
            fct = const_pool.tile([KJ, OUT], wdt)
            nc.sync.dma_start(fct[:], w16_ap[:, W16_FCT : W16_FCT + OUT])
            fcb = const_pool.tile([OUT, 1], f32)
            nc.sync.dma_start(fcb[:], w32_ap[0:OUT, 0:1])
            x0m = const_pool.tile([NPM, NMOD], f32)
            nc.sync.dma_start(x0m[:], w32_ap[:, 1 : 1 + NMOD])
            zeros = const_pool.tile([NPM, HFREE], wdt)
            nc.vector.memset(zeros[:], 0.0)
            zf32 = const_pool.tile([NPM, HB], f32)
            nc.vector.memset(zf32[:], 0.0)

            pre_a0 = const_pool.tile([NPM, HFREE], f32)
            pre_a1 = const_pool.tile([NPM, HFREE], f32)
            pre_b0 = const_pool.tile([NPM, HFREE], f32)
            pre_b1 = const_pool.tile([NPM, HFREE], f32)
            pres = [[pre_a0, pre_a1], [pre_b0, pre_b1]]
            rd_a0 = const_pool.tile([KJ, HFREE], wdt)
            rd_a1 = const_pool.tile([KJ, HFREE], wdt)
            rd_b0 = const_pool.tile([KJ, HFREE], wdt)
            rd_b1 = const_pool.tile([KJ, HFREE], wdt)
            rds = [[rd_a0, rd_a1], [rd_b0, rd_b1]]
            # data rows that only ever meet zero weights: zero the whole
            # tile once (memset must start at partition 0); the ones row
            # (drives Bb) in module-slice 0 arrives by DMA (no partition-
            # start restriction).
            for h in range(NH):
                for p in range(2):
                    nc.vector.memset(rds[h][p][:], 0.0)
                    nc.sync.dma_start(
                        rds[h][p][KJ - 1 : KJ, 0:HB],
                        w16_ap[0:1, W16_ONES : W16_ONES + HB],
                    )
            if variant in ("no_chain", "ew_only"):
                dump_r = const_pool.tile([NPM, HFREE], wdt)
                dump_p = const_pool.tile([NPM, HFREE], f32)
            if variant == "ew_only":
                psc_pool = ctx.enter_context(
                    tc.tile_pool(name="psc", bufs=1, space="PSUM")
                )
                ew_ps0 = psc_pool.tile([128, 512], f32)
                ew_ps1 = psc_pool.tile([128, 512], f32)
                nc.vector.memset(ew_ps0[:], 0.25)
                nc.vector.memset(ew_ps1[:], 0.25)
                ew_pss = [ew_ps0, ew_ps1]

            rep_ctx = (
                tc.For_i(0, n_repeat, 1)
                if n_repeat > 1
                else contextlib.nullcontext()
            )
            with rep_ctx:
                for h in range(NH):
                    # pre0 = x0 broadcast across batch: 3 tensor_scalar
                    # adds (0 + x0[:,m]) replace a 64x-redundant 1.2MB
                    # sin_pre upload
                    for m in range(NMOD):
                        nc.vector.tensor_scalar_add(
                            pres[h][0][:, m * HB : (m + 1) * HB],
                            zf32[:], x0m[:, m : m + 1],
                        )
                    # r = retanh(pre) identically, so only pre is carried
                    # state; recompute r here
                    th0 = tmp_pool.tile([NPM, HFREE], wdt, tag=f"init{h}")
                    nc.scalar.activation(
                        th0[:], pres[h][0][:],
                        mybir.ActivationFunctionType.Tanh,
                    )
                    nc.vector.tensor_tensor(
                        rds[h][0][0:NPM, :], th0[:], zeros[:],
                        op=mybir.AluOpType.max,
                    )
                    stage_data(0, h, rds[h][0])
                    if n_steps > 1:
                        stage_data(1, h, rds[h][1])
                    if variant in ("no_chain", "pe_only"):
                        nc.vector.tensor_tensor(
                            rds[h][1][0:NPM, :], th0[:], zeros[:],
                            op=mybir.AluOpType.max,
                        )

                ybuf = None
                for t in range(n_steps):
                    s = t - 1          # step whose y this group computes
                    if s % ch == 0:
                        ybuf = yout_pool.tile([OUT, ch * BS], ydt, tag="ybuf")
                    for h in range(NH):
                        pre_cur = pres[h][t % 2]
                        pre_nxt = pres[h][(t + 1) % 2]
                        rd = rds[h][t % 2]
                        rd_nxt = rds[h][(t + 1) % 2]

                        if variant == "ew_only":
                            ps = ew_pss[h]
                        else:
                            ps = ps_pool.tile([128, 512], f32, tag=f"ps{h}")
                        for k in range(NMOD):
                            if variant == "ew_only":
                                break
                            rk = rd[0:KJ, k * HB : (k + 1) * HB]
                            for m in range(NMOD):
                                nc.tensor.matmul(
                                    ps[0:NPM, m * HB : (m + 1) * HB],
                                    jt[:, (k * NMOD + m) * NPM : (k * NMOD + m) * NPM + NPM],
                                    rk,
                                    start=(k == 0 and m == 0),
                                    stop=False,
                                )
                        # y_{t-1}: r_{t-1} is rd's r rows (relu_t writes
                        # rd_nxt, not rd). Last in the group, carries stop.
                        if variant != "ew_only":
                            nc.tensor.matmul(
                                ps[0:OUT, HFREE : HFREE + HB],
                                fct[:],
                                rd[0:KJ, 0:HB],
                                start=False,
                                stop=True,
                            )
                        # --- elementwise (overlaps the other half's PE) ---
                        if variant == "pe_only":
                            if t + 2 < n_steps:
                                stage_data(t + 2, h, rd)
                            continue
                        # Euler step on DVE: pre' = 0.9*pre + 0.1*(...)
                        # (the 0.1 is folded into jt; saves PE the ident
                        # matmuls + their LDWEIGHTS). The 0.9*pre scaling
                        # runs off the critical path (no PSUM dependency).
                        th = tmp_pool.tile([NPM, HFREE], wdt, tag=f"th{h}")
                        sc = tmp_pool.tile([NPM, HFREE], f32, tag=f"sc{h}")
                        if variant in ("no_chain", "ew_only"):
                            nc.vector.tensor_scalar_mul(
                                sc[:], pres[h][0][:], 0.9
                            )
                            nc.vector.tensor_tensor(
                                dump_p[:], sc[:], ps[0:NPM, 0:HFREE],
                                op=mybir.AluOpType.add,
                            )
                            nc.scalar.activation(
                                th[:], dump_p[:],
                                mybir.ActivationFunctionType.Tanh,
                            )
                            nc.vector.tensor_tensor(
                                dump_r[:], th[:], zeros[:],
                                op=mybir.AluOpType.max,
                            )
                        else:
                            nc.vector.tensor_scalar_mul(
                                sc[:], pre_cur[:], 0.9
                            )
                            nc.vector.tensor_tensor(
                                pre_nxt[:], sc[:], ps[0:NPM, 0:HFREE],
                                op=mybir.AluOpType.add,
                            )
                            nc.scalar.activation(
                                th[:], pre_nxt[:],
                                mybir.ActivationFunctionType.Tanh,
                            )
                            # r <- relu(tanh) via TT-max (2x DVE mode)
                            nc.vector.tensor_tensor(
                                rd_nxt[0:NPM, :], th[:], zeros[:],
                                op=mybir.AluOpType.max,
                            )
                        if t > 0:
                            nc.vector.tensor_scalar_add(
                                ybuf[:, (s % ch) * BS + h * HB : (s % ch) * BS + (h + 1) * HB],
                                ps[0:OUT, HFREE : HFREE + HB],
                                fcb[:],
                            )
                        # stage d_{t+2} for this parity tile (WAR: this
                        # group's J matmuls; ~2 steps of slack).
                        if t + 2 < n_steps:
                            stage_data(t + 2, h, rd)
                    if variant != "pe_only" and t > 0 and s % ch == ch - 1:
                        nc.sync.dma_start(
                            y_ap[:, (s - ch + 1) * BS : (s + 1) * BS], ybuf[:]
                        )

                # trailing: y of the last step, per half
                s = n_steps - 1
                if s % ch == 0:
                    ybuf = yout_pool.tile([OUT, ch * BS], ydt, tag="ybuf")
                for h in range(NH):
                    ps = ps_pool.tile([128, 512], f32, tag=f"ps{h}")
                    nc.tensor.matmul(
                        ps[0:OUT, HFREE : HFREE + HB],
                        fct[:],
                        rds[h][n_steps % 2][0:KJ, 0:HB],
                        start=True,
                        stop=True,
                    )
                    nc.vector.tensor_scalar_add(
                        ybuf[:, (s % ch) * BS + h * HB : (s % ch) * BS + (h + 1) * HB],
                        ps[0:OUT, HFREE : HFREE + HB],
                        fcb[:],
                    )
                nc.sync.dma_start(
                    y_ap[:, (s - s % ch) * BS : (s + 1) * BS],
                    ybuf[:, : (s % ch + 1) * BS],
                )

                # final state out (for phase pipelining)
                for h in range(NH):
                    nc.sync.dma_start(hs(sout_pre_ap, h), pres[h][n_steps % 2][:])

    nc.compile()
    return nc


def _get_program(n_steps: int, n_repeat: int = 1, variant: str = "full"):
    key = (n_steps, W_DT, Y_DT, n_repeat, NH, variant)
    if key not in _BUILD_CACHE:
        _BUILD_CACHE[key] = _build_program(n_steps, n_repeat, variant)
    return _BUILD_CACHE[key]


def _plan_chunks(n_steps: int):
    phases = PHASES
    while phases > 1 and n_steps < phases * MIN_PHASE_STEPS:
        phases -= 1
    base = n_steps // phases
    rem = n_steps - base * phases
    # equal chunks when divisible (one compiled program serves all phases)
    return [base + (1 if i < rem else 0) for i in range(phases)]


def _rep8(a):
    return np.ascontiguousarray(
        np.broadcast_to(a, (N_CORES, *a.shape))
    ).reshape(N_CORES * a.shape[0], a.shape[1])


def _prep_arrays(data, J, I, S, Bb, x0, fc_w, fc_b, chunks):
    """Build the global (axis-0 concatenated) input arrays for shard_map."""
    wnp = _w_np()
    f32 = np.float32

    n_steps = sum(chunks)
    dat_f = np.asarray(data, f32)[:n_steps]
    if D_DT == "int8":
        # amax via two reductions (no 43MB abs temporary)
        amax = max(float(dat_f.max()), -float(dat_f.min()))
        dsc = np.float32(max(amax / 127.0, 1e-30))
    else:
        dsc = np.float32(1.0)

    Jp = 0.1 * np.asarray(J, f32)
    Ip = 0.1 * dsc * np.asarray(I, f32)   # data wire scale rides the weights
    Sp = 0.1 * dsc * np.asarray(S, f32)
    Bbp = 0.1 * np.asarray(Bb, f32)       # Bb enters via the ones row: unscaled

    # jt: rows 0:100 = J'[m,k].T ; rows 100:122 = input weights on k==0
    jt = np.zeros((KJ, 9, NPM), f32)
    for k in range(NMOD):
        for m in range(NMOD):
            blk = Jp[m * NPM : (m + 1) * NPM, k * NPM : (k + 1) * NPM]
            jt[:NPM, k * NMOD + m, :NPM] = blk.T
            if k == 0:
                jt[NPM : NPM + NF, k * NMOD + m, :NPM] = (
                    Ip[m * NPM : (m + 1) * NPM, :].T
                )
                jt[NPM + NF, k * NMOD + m, :NPM] = Sp[m * NPM : (m + 1) * NPM, 0]
                jt[NPM + NF + 1, k * NMOD + m, :NPM] = (
                    Bbp[m * NPM : (m + 1) * NPM, 0]
                )

    ysc = Y_SCALE if Y_DT == "int8" else 1.0  # y wire scale folds into fc
    w16 = np.zeros((KJ, W16_COLS), f32)
    w16[:, : 9 * NPM] = jt.reshape(KJ, 9 * NPM)
    w16[:NPM, W16_FCT : W16_FCT + OUT] = ysc * np.asarray(fc_w, f32).T
    w16[0, W16_ONES : W16_ONES + HB] = 1.0
    w16 = w16.astype(wnp)

    w32 = np.zeros((NPM, 1 + NMOD), f32)
    w32[:OUT, 0] = ysc * np.asarray(fc_b, f32)
    w32[:, 1:] = np.asarray(x0, f32).reshape(NMOD, NPM).T

    # din: per phase [8*21, steps*128] — core-major, then t-major
    if D_DT == "int8":
        # |x|/dsc <= 127 by construction of dsc, so no clip needed;
        # rint == round (both half-even), minus round()'s overhead
        tmp = dat_f * np.float32(1.0 / dsc)
        np.rint(tmp, out=tmp)
        dat = tmp.astype(np.int8)
    else:
        dat = dat_f.astype(wnp)           # [T, 21, B]
    dins, t0 = [], 0
    for c in chunks:
        dins.append(
            np.ascontiguousarray(
                np.transpose(
                    dat[t0 : t0 + c].reshape(c, KDATA, N_CORES, BS),
                    (2, 1, 0, 3),
                )
            ).reshape(N_CORES * KDATA, c * BS)
        )
        t0 += c

    return {
        "din": dins,
        "w16": _rep8(w16),
        "w32": _rep8(w32),
    }


class _Runner:
    """Persistent jitted shard_map callable for one compiled program."""

    IN_ORDER = ("din", "w16", "w32")

    def __init__(self, nc):
        import jax
        import jax.numpy as jnp
        from jax.sharding import Mesh, PartitionSpec
        from jax.experimental.shard_map import shard_map
        from concourse.bass2jax import (
            _bass_exec_p,
            install_neuronx_cc_hook,
            partition_id_tensor,
        )

        install_neuronx_cc_hook()
        self.nc = nc
        partition_name = (
            nc.partition_id_tensor.name if nc.partition_id_tensor else None
        )

        in_names, out_names, out_avals, zero_shapes = [], [], [], []
        for alloc in nc.m.functions[0].allocations:
            if not isinstance(alloc, mybir.MemoryLocationSet):
                continue
            name = alloc.memorylocations[0].name
            if alloc.kind == "ExternalInput":
                if name != partition_name:
                    in_names.append(name)
            elif alloc.kind == "ExternalOutput":
                np_dt = mybir.dt.np(alloc.dtype)
                out_avals.append(
                    jax.core.ShapedArray(tuple(alloc.tensor_shape), np_dt)
                )
                out_names.append(name)
                zero_shapes.append((tuple(alloc.tensor_shape), np_dt))
        assert tuple(in_names) == self.IN_ORDER, in_names
        assert out_names[0] == "y", out_names
        self.in_names = in_names
        self.out_names = out_names

        n_params = len(in_names)
        n_outs = len(out_names)
        all_in_names = list(in_names) + list(out_names)
        if partition_name is not None:
            all_in_names.append(partition_name)

        def _body(*args):
            operands = list(args)
            if partition_name is not None:
                operands.append(partition_id_tensor())
            outs = _bass_exec_p.bind(
                *operands,
                out_avals=tuple(out_avals),
                in_names=tuple(all_in_names),
                out_names=tuple(out_names),
                lowering_input_output_aliases=(),
                sim_require_finite=True,
                sim_require_nnan=True,
                nc=nc,
            )
            return tuple(outs)

        devices = jax.devices()[:N_CORES]
        mesh = Mesh(np.asarray(devices), ("core",))
        in_specs = (PartitionSpec("core"),) * (n_params + n_outs)
        out_specs = (PartitionSpec("core"),) * n_outs
        self.mesh = mesh
        self.sharded = jax.jit(
            shard_map(
                _body, mesh=mesh, in_specs=in_specs, out_specs=out_specs,
                check_rep=False,
            ),
            keep_unused=True,
        )
        # device-resident zero output buffers, reused every call
        self.zeros = [
            jnp.zeros((N_CORES * shp[0], *shp[1:]), dt)
            for shp, dt in zero_shapes
        ]

    def __call__(self, din, w16, w32):
        return self.sharded(din, w16, w32, *self.zeros)


def _get_runner(n_steps: int, n_repeat: int = 1, variant: str = "full"):
    key = (n_steps, W_DT, Y_DT, n_repeat, NH, variant)
    if key not in _RUNNER_CACHE:
        _RUNNER_CACHE[key] = _Runner(_get_program(n_steps, n_repeat, variant))
    return _RUNNER_CACHE[key]


def _convert_shard(dst_f32, qa, c, n_steps, t_off):
    dst = dst_f32[t_off : t_off + n_steps, c * BS : (c + 1) * BS, :]
    if Y_DT == "int8":
        v = qa.view(np.int8).reshape(OUT, n_steps, BS).transpose(1, 2, 0)
        np.multiply(
            v, np.float32(1.0 / Y_SCALE), out=dst, casting="unsafe"
        )
    else:
        v = qa.view(ml_dtypes.bfloat16).reshape(OUT, n_steps, BS)
        dst[...] = v.transpose(1, 2, 0)


def _gather_y(y_global: np.ndarray, n_steps: int) -> np.ndarray:
    """[8*OUT, n_steps*BS] wire format -> [n_steps, B, OUT] f32."""
    final = np.empty((n_steps, B, OUT), np.float32)
    per = np.asarray(y_global).reshape(N_CORES, OUT, n_steps * BS)
    for c in range(N_CORES):
        _convert_shard(final, per[c], c, n_steps, 0)
    return final


def run_sharded(inputs: dict, n_steps: int = T):
    """Compile (cached), run on 8 cores phase-pipelined, return [T, B, OUT]."""
    from concurrent.futures import ThreadPoolExecutor
    import jax
    from jax.sharding import NamedSharding, PartitionSpec

    chunks = _plan_chunks(n_steps)
    runners = [_get_runner(c) for c in chunks]
    arrays = _prep_arrays(chunks=chunks, **inputs)

    # multi-phase: weights ride the tunnel once, then stay device-resident
    if len(chunks) > 1:
        spec = NamedSharding(runners[0].mesh, PartitionSpec("core"))
        w16 = jax.device_put(arrays["w16"], spec)
        w32 = jax.device_put(arrays["w32"], spec)
    else:
        w16, w32 = arrays["w16"], arrays["w32"]

    ys = []
    for i, c in enumerate(chunks):
        outs = runners[i](arrays["din"][i], w16, w32)
        ys.append(outs[0])

    # all phases dispatched async; the tunnel serializes shard fetches,
    # so convert each shard on a worker thread while the next downloads
    final = np.empty((n_steps, B, OUT), np.float32)
    with ThreadPoolExecutor(2) as ex:
        futs, t_off = [], 0
        for i, c in enumerate(chunks):
            shards = sorted(
                ys[i].addressable_shards, key=lambda s: s.index[0].start
            )
            for sh in shards:
                sh.data.copy_to_host_async()
            for cc, sh in enumerate(shards):
                futs.append(
                    ex.submit(
                        _convert_shard, final, np.asarray(sh.data), cc, c, t_off
                    )
                )
            t_off += c
        for f in futs:
            f.result()
    return final


def kernel(data, J, I, S, Bb, x0, fc_w, fc_b):
    return run_sharded(
        dict(data=data, J=J, I=I, S=S, Bb=Bb, x0=x0, fc_w=fc_w, fc_b=fc_b)
    )
